# revision 13
# baseline (speedup 1.0000x reference)
"""CIN (xDeepFM CompressedInteractionNetwork) forward on 8 TRN2 NeuronCores.

v5: fully decoupled software pipeline. v4 showed the loop-carried chain
L0(i)->h1->z1-build->convert->L1(i) (same PE iteration) sets a ~16us
cycle even with balanced engines; v5 runs L1 one iteration late --
PE iteration i = [L0(i), L2(i-3), L1(i-1)] -- so every cross-engine
handoff (h1->z1 builds, converts, h2->z2 builds) has at least a full
iteration of slack and the wall drops to the PE's own per-iteration
work (~11.8us/tile at NB=4).

Measured HW rates: DR fp8 matmul 157TF/s (2x bf16), DVE TT bf16-out
0.556ns/elem / fp8-out 1.19, Act convert ~1.2us/quarter, GpSimd TT
2.26ns/elem. fp8 conversion costs more engine-time than it saves on the
PE, so it's pure load-balancing; the LP-optimal split:

  - L1 k-subtiles: A = f0:8 (DVE bf16 build + ScalarE per-quarter fp8
    convert, DR matmuls), S = f8:32 stay-bf16 (plain matmuls).
  - L2 k-subtiles: D = f0:16 (DVE direct-fp8 build, 2 iters of slack),
    P = f16:32 (GpSimd bf16 build + ScalarE convert -- the slow Pool
    engine only carries slack-tolerant work).
  - L0 exact bf16 from host-precomputed symmetrized pair products
    (528 unordered pairs padded to 768 = 6x128).
  - fp8 weights x64 (un-scaled via the activation's 1/64); stay-class
    bf16 weights also x64 so one PSUM group mixes dtypes.
  - schedule of tile c: h1(c)@iter c, z1(c)@iter c, L1(c)@iter c+1,
    h2(c)@iter c+1, z2-Pool(c)@iter c+1 tail, z2-DVE(c)@iter c+2 head,
    z2 convert@iter c+2, L2(c)@iter c+3. xr broadcast lives 3 iters
    (bufs=4); every rotating tile is double-buffered.
  - pooling reduces on DVE; final FC on host.
"""

import sys

sys.path.insert(0, "/opt/trn_rl_repo")

import numpy as np
import ml_dtypes
from contextlib import ExitStack

N_CORES = 8
B = 4096
F = 32
E = 64
BC = B // N_CORES  # 512 batch elements per core
NB = 4             # batch elements per tile
COLS = NB * E      # 256 matmul columns per tile
NT = BC // NB      # 128 tiles per core
O = 256            # conv out channels per layer
NP0 = 5            # L0 k-subtiles: 528 pairs padded to 640 = 5*128
S12 = 64.0         # fp8 weight scale for L1/L2

A1 = 20            # L1: A = f0:A1 (fp8 via convert), S = rest (bf16)
D2 = 4             # L2: D = f0:D2 (direct fp8)
A2 = 20            # L2: A = f[D2:A2] (bf16 + convert), S = rest (bf16)

_CACHE = {}


def _build(n_tiles=NT):
    import concourse.bass as bass  # noqa: F401
    import concourse.mybir as mybir
    import concourse.tile as tile
    from concourse import bacc

    dt = mybir.dt
    AF = mybir.ActivationFunctionType
    ALU = mybir.AluOpType
    AX = mybir.AxisListType
    DR = mybir.MatmulPerfMode.DoubleRow

    nc = bacc.Bacc("TRN2", target_bir_lowering=False, debug=False,
                   num_devices=N_CORES)

    z0q = nc.declare_dram_parameter("z0q", [n_tiles, 128, NP0 * COLS],
                                    dt.bfloat16, isOutput=False)
    xtile = nc.declare_dram_parameter("xtile", [n_tiles, F, COLS],
                                      dt.bfloat16, isOutput=False)
    w0t = nc.declare_dram_parameter("w0t", [128, NP0 * O], dt.bfloat16,
                                    isOutput=False)
    w1f8 = nc.declare_dram_parameter("w1f8", [128, 2 * A1 * 128],
                                     dt.float8e4, isOutput=False)
    w1bf = nc.declare_dram_parameter("w1bf", [128, 2 * (F - A1) * 128],
                                     dt.bfloat16, isOutput=False)
    w2f8 = nc.declare_dram_parameter("w2f8", [128, 2 * A2 * 128],
                                     dt.float8e4, isOutput=False)
    w2bf = nc.declare_dram_parameter("w2bf", [128, 2 * (F - A2) * 128],
                                     dt.bfloat16, isOutput=False)
    b0 = nc.declare_dram_parameter("b0", [O], dt.float32, isOutput=False)
    b1 = nc.declare_dram_parameter("b1", [O], dt.float32, isOutput=False)
    b2 = nc.declare_dram_parameter("b2", [O], dt.float32, isOutput=False)
    pout = nc.declare_dram_parameter("pout", [4, 128, n_tiles * NB],
                                     dt.float32, isOutput=True)

    with ExitStack() as ctx:
        tc = ctx.enter_context(tile.TileContext(nc))
        const = ctx.enter_context(tc.tile_pool(name="const", bufs=1))

        lw0 = const.tile([128, NP0, O], dt.bfloat16)
        lw1f = const.tile([128, 2, A1, 128], dt.float8e4)
        lw1b = const.tile([128, 2, F - A1, 128], dt.bfloat16)
        lw2f = const.tile([128, 2, A2, 128], dt.float8e4)
        lw2b = const.tile([128, 2, F - A2, 128], dt.bfloat16)
        bias0 = const.tile([128, 2], dt.float32)
        bias1 = const.tile([128, 2], dt.float32)
        bias2 = const.tile([128, 2], dt.float32)

        P0 = const.tile([128, n_tiles * NB], dt.float32)
        P1 = const.tile([128, n_tiles * NB], dt.float32)
        P2a = const.tile([128, n_tiles * NB], dt.float32)
        P2b = const.tile([128, n_tiles * NB], dt.float32)

        z0_pool = ctx.enter_context(tc.tile_pool(name="z0", bufs=5))
        xr_pool = ctx.enter_context(tc.tile_pool(name="xr", bufs=4))
        z1_pool = ctx.enter_context(tc.tile_pool(name="z1", bufs=2))
        z2_pool = ctx.enter_context(tc.tile_pool(name="z2", bufs=2))
        h_pool = ctx.enter_context(tc.tile_pool(name="h", bufs=2))
        r_pool = ctx.enter_context(tc.tile_pool(name="r", bufs=2))
        psum_pool = ctx.enter_context(tc.tile_pool(name="ps", bufs=8, space="PSUM"))

        z0t = [None] * n_tiles
        xrt = [None] * n_tiles
        z1t = [None] * n_tiles
        z2dt = [None] * n_tiles
        z2pqt = [None] * n_tiles
        z2pbt = [None] * n_tiles
        h2t = [None] * n_tiles
        r1t = [None] * n_tiles

        def emit_z0dma(t):
            z0 = z0_pool.tile([128, NP0, COLS], dt.bfloat16)
            nc.sync.dma_start(z0[:].rearrange("p g c -> p (g c)"), z0q.ap()[t])
            z0t[t] = z0

        def emit_xrdma(t):
            xr = xr_pool.tile([128, F, COLS], dt.bfloat16)
            src = xtile.ap()[t].unsqueeze(0).broadcast_to([128, F, COLS])
            nc.sync.dma_start(xr[:], src)
            xrt[t] = xr

        def reduce_into(P, t, r_t):
            nc.vector.tensor_reduce(
                P[:, t * NB:(t + 1) * NB],
                r_t[:].rearrange("p (b e) -> p b e", e=E), AX.X, ALU.add)

        def hbc(h, n):
            return h[:].unsqueeze(1).broadcast_to([128, n, COLS])

        # ---- preamble ----
        K0 = 2          # L0 runs K0 tiles ahead of the rest of the pipeline
        h1t = [None] * n_tiles
        emit_z0dma(0)
        emit_z0dma(1)
        emit_z0dma(2)
        emit_xrdma(0)
        nc.sync.dma_start(lw0[:], w0t.ap().rearrange("p (g o) -> p g o", o=O))
        nc.sync.dma_start(bias0[:], b0.ap().rearrange("(m p) -> p m", p=128))
        w1fv = w1f8.ap().rearrange("p (m x) -> p m x", m=2)
        w1bv = w1bf.ap().rearrange("p (m x) -> p m x", m=2)
        w2fv = w2f8.ap().rearrange("p (m x) -> p m x", m=2)
        w2bv = w2bf.ap().rearrange("p (m x) -> p m x", m=2)
        for m in (1, 0):
            nc.sync.dma_start(lw1f[:, m].rearrange("p g o -> p (g o)"), w1fv[:, m])
            nc.sync.dma_start(lw1b[:, m].rearrange("p g o -> p (g o)"), w1bv[:, m])
        nc.sync.dma_start(bias1[:], b1.ap().rearrange("(m p) -> p m", p=128))
        for m in (0, 1):
            nc.sync.dma_start(lw2f[:, m].rearrange("p g o -> p (g o)"), w2fv[:, m])
            nc.sync.dma_start(lw2b[:, m].rearrange("p g o -> p (g o)"), w2bv[:, m])
        nc.sync.dma_start(bias2[:], b2.ap().rearrange("(m p) -> p m", p=128))

        for i in range(n_tiles + 3):
            if i + 1 < n_tiles:
                emit_xrdma(i + 1)
            if i + K0 + 1 < n_tiles:
                emit_z0dma(i + K0 + 1)

            # -- DVE head: z2(i-2) builds (h2(i-2) one iter old) --
            c2 = i - 2
            if 0 <= c2 < n_tiles:
                z2d = z2_pool.tile([128, D2, COLS], dt.float8e4,
                                   name="z2d", tag="z2d")
                nc.vector.tensor_tensor(
                    z2d[:], xrt[c2][:, 0:D2, :], hbc(h2t[c2], D2), ALU.mult)
                z2ab = z2_pool.tile([128, A2 - D2, COLS], dt.bfloat16,
                                    name="z2ab", tag="z2ab")
                nc.vector.tensor_tensor(
                    z2ab[:], xrt[c2][:, D2:A2, :], hbc(h2t[c2], A2 - D2),
                    ALU.mult)
                z2s = z2_pool.tile([128, F - A2, COLS], dt.bfloat16,
                                   name="z2s", tag="z2s")
                nc.vector.tensor_tensor(
                    z2s[:], xrt[c2][:, A2:, :], hbc(h2t[c2], F - A2), ALU.mult)
                z2dt[c2] = (z2d, z2ab, z2s)

            l0_tiles = list(range(0, K0 + 1)) if i == 0 else (
                [i + K0] if 0 < i + K0 < n_tiles else [])
            for t0 in l0_tiles:
                # -- PE: L0(t0) bf16 exact (K0 tiles ahead); m=1 first --
                ps0 = {m: psum_pool.tile([128, COLS], dt.float32,
                                         name=f"ps0{m}", tag="ps",
                                         padded_shape=[128, 512])
                       for m in (1, 0)}
                for m in (1, 0):
                    for g in range(NP0):
                        nc.tensor.matmul(
                            ps0[m][:], lw0[:, g, m * 128:(m + 1) * 128],
                            z0t[t0][:, g, :], start=(g == 0), stop=(g == NP0 - 1))
                h1 = h_pool.tile([128, COLS], dt.bfloat16, name="h1", tag="h1",
                                 bufs=K0 + 2)
                nc.scalar.activation(h1[:], ps0[1][:], AF.Relu, bias=bias0[:, 1:2])
                h1t[t0] = h1
                r0 = r_pool.tile([128, COLS], dt.bfloat16, name="r0", tag="r0")
                nc.scalar.activation(r0[:], ps0[0][:], AF.Relu, bias=bias0[:, 0:1])
                reduce_into(P0, t0, r0)

            if i < n_tiles:
                # -- DVE: z1(i) builds; Act converts per quarter --
                h1 = h1t[i]
                xr = xrt[i]
                z1ab = z1_pool.tile([128, A1, COLS], dt.bfloat16,
                                    name="z1ab", tag="z1ab")
                z1aq = z1_pool.tile([128, A1, COLS], dt.float8e4,
                                    name="z1aq", tag="z1aq")
                nc.vector.tensor_tensor(
                    z1ab[:], xr[:, 0:A1, :], hbc(h1, A1), ALU.mult)
                nc.scalar.activation(z1aq[:], z1ab[:], AF.Copy)
                z1s = z1_pool.tile([128, F - A1, COLS], dt.bfloat16,
                                   name="z1s", tag="z1s")
                nc.vector.tensor_tensor(
                    z1s[:], xr[:, A1:, :], hbc(h1, F - A1), ALU.mult)
                z1t[i] = (z1aq, z1s)

            # -- Act: z2-A(i-2) convert --
            if 0 <= c2 < n_tiles:
                z2aq = z2_pool.tile([128, A2 - D2, COLS], dt.float8e4,
                                    name="z2aq", tag="z2aq")
                nc.scalar.activation(z2aq[:], z2dt[c2][1][:], AF.Copy)
                z2pqt[c2] = z2aq

            if i >= 3 and i - 3 < n_tiles:
                # -- PE: L2(i-3): D then P, all fp8 DR --
                cc = i - 3
                ps2 = {m: psum_pool.tile([128, COLS], dt.float32,
                                         name=f"ps2{m}", tag="ps",
                                         padded_shape=[128, 512])
                       for m in (0, 1)}
                z2d_, _, z2s_ = z2dt[cc]
                for m in (0, 1):
                    nops = A2 // 2 + (F - A2)
                    k = 0
                    for g in range(D2 // 2):
                        nc.tensor.matmul(
                            ps2[m][:], lw2f[:, m, 2 * g:2 * g + 2, :],
                            z2d_[:, 2 * g:2 * g + 2, :],
                            start=(k == 0), stop=False, perf_mode=DR)
                        k += 1
                    for g in range((A2 - D2) // 2):
                        nc.tensor.matmul(
                            ps2[m][:], lw2f[:, m, D2 + 2 * g:D2 + 2 * g + 2, :],
                            z2pqt[cc][:, 2 * g:2 * g + 2, :],
                            start=False, stop=(k == nops - 1), perf_mode=DR)
                        k += 1
                    for s in range(F - A2):
                        nc.tensor.matmul(
                            ps2[m][:], lw2b[:, m, s, :], z2s_[:, s, :],
                            start=False, stop=(k == nops - 1))
                        k += 1
                r2a = r_pool.tile([128, COLS], dt.bfloat16, name="r2a",
                                  tag="r2a")
                nc.scalar.activation(r2a[:], ps2[0][:], AF.Relu,
                                     bias=bias2[:, 0:1], scale=1.0 / S12)
                r2b = r_pool.tile([128, COLS], dt.bfloat16, name="r2b",
                                  tag="r2b")
                nc.scalar.activation(r2b[:], ps2[1][:], AF.Relu,
                                     bias=bias2[:, 1:2], scale=1.0 / S12)

            c1 = i - 1
            if 0 <= c1 < n_tiles:
                # -- PE: L1(i-1): A (fp8 DR) then S (bf16); m=1 first --
                z1aq, z1s = z1t[c1]
                nmm = A1 // 2 + (F - A1)
                ps1 = {m: psum_pool.tile([128, COLS], dt.float32,
                                         name=f"ps1{m}", tag="ps",
                                         padded_shape=[128, 512])
                       for m in (1, 0)}
                for m in (1, 0):
                    k = 0
                    for g in range(A1 // 2):
                        nc.tensor.matmul(
                            ps1[m][:], lw1f[:, m, 2 * g:2 * g + 2, :],
                            z1aq[:, 2 * g:2 * g + 2, :],
                            start=(k == 0), stop=(k == nmm - 1), perf_mode=DR)
                        k += 1
                    for s in range(F - A1):
                        nc.tensor.matmul(
                            ps1[m][:], lw1b[:, m, s, :], z1s[:, s, :],
                            start=False, stop=(k == nmm - 1))
                        k += 1
                h2 = h_pool.tile([128, COLS], dt.bfloat16, name="h2", tag="h2")
                nc.scalar.activation(h2[:], ps1[1][:], AF.Relu,
                                     bias=bias1[:, 1:2], scale=1.0 / S12)
                r1 = r_pool.tile([128, COLS], dt.bfloat16, name="r1", tag="r1")
                nc.scalar.activation(r1[:], ps1[0][:], AF.Relu,
                                     bias=bias1[:, 0:1], scale=1.0 / S12)
                h2t[c1] = h2
                r1t[c1] = r1

            # -- DVE reduces (all inputs >= same-iteration-early) --
            if i >= 3 and i - 3 < n_tiles:
                reduce_into(P2a, i - 3, r2a)
                reduce_into(P2b, i - 3, r2b)
            if i >= 2 and i - 2 < n_tiles:
                reduce_into(P1, i - 2, r1t[i - 2])

            if i == n_tiles - 1:
                nc.sync.dma_start(pout.ap()[0], P0[:])
            if i == n_tiles + 1:
                nc.sync.dma_start(pout.ap()[1], P1[:])

        nc.sync.dma_start(pout.ap()[2], P2a[:])
        nc.sync.dma_start(pout.ap()[3], P2b[:])

    nc.compile()
    return nc


def _pair_indices():
    ia = [f for f in range(F)]
    ib = [f for f in range(F)]
    for f1 in range(F):
        for f2 in range(f1 + 1, F):
            ia.append(f1)
            ib.append(f2)
    return np.asarray(ia, np.int64), np.asarray(ib, np.int64)


def _prep_inputs(x, w0, b0, w1, b1, w2, b2, fc_w, fc_b):
    bf16 = ml_dtypes.bfloat16
    f8 = ml_dtypes.float8_e4m3
    xb = np.asarray(x, np.float32).astype(bf16)

    ia, ib = _pair_indices()
    iap = np.zeros(NP0 * 128, np.int64); iap[:528] = ia
    ibp = np.zeros(NP0 * 128, np.int64); ibp[:528] = ib

    w0f = np.asarray(w0, np.float32).reshape(O, F, F)
    w0s = np.zeros((O, NP0 * 128), np.float32)
    w0s[:, :528] = w0f[:, ia, ib]
    off = ia != ib
    w0s[:, :528][:, off] += w0f[:, ib[off], ia[off]]
    w0t = np.ascontiguousarray(
        w0s.T.reshape(NP0, 128, O).transpose(1, 0, 2).reshape(128, NP0 * O)
    ).astype(bf16)

    def wsplit(w, nf8):
        ws = np.asarray(w, np.float32).T.reshape(F, 128, 2, 128) * S12
        wf = ws[:nf8].astype(f8).transpose(1, 2, 0, 3)
        out = [np.ascontiguousarray(wf.reshape(128, -1))]
        if nf8 < F:
            wb = ws[nf8:].astype(bf16).transpose(1, 2, 0, 3)
            out.append(np.ascontiguousarray(wb.reshape(128, -1)))
        return out

    w1f, w1b_ = wsplit(w1, A1)
    w2f, w2b_ = wsplit(w2, A2)
    common = {
        "w0t": w0t, "w1f8": w1f, "w1bf": w1b_, "w2f8": w2f, "w2bf": w2b_,
        "b0": np.ascontiguousarray(np.asarray(b0, np.float32)),
        "b1": np.ascontiguousarray(np.asarray(b1, np.float32)),
        "b2": np.ascontiguousarray(np.asarray(b2, np.float32)),
    }
    in_maps = []
    for c in range(N_CORES):
        m = dict(common)
        xc = xb[c * BC:(c + 1) * BC]
        xt = np.ascontiguousarray(
            xc.reshape(NT, NB, F, E).transpose(0, 2, 1, 3).reshape(NT, F, COLS))
        m["xtile"] = xt
        xf = xt.astype(np.float32)
        g = xf[:, iap, :] * xf[:, ibp, :]
        m["z0q"] = np.ascontiguousarray(
            g.reshape(NT, NP0, 128, COLS).transpose(0, 2, 1, 3)
             .reshape(NT, 128, NP0 * COLS).astype(bf16))
        in_maps.append(m)
    return in_maps


def kernel(x, w0, b0, w1, b1, w2, b2, fc_w, fc_b, **kw):
    from concourse.bass_utils import run_bass_kernel_spmd

    if "nc" not in _CACHE:
        _CACHE["nc"] = _build()
    nc = _CACHE["nc"]
    in_maps = _prep_inputs(x, w0, b0, w1, b1, w2, b2, fc_w, fc_b)
    res = run_bass_kernel_spmd(nc, in_maps, list(range(N_CORES)))
    fcw = np.asarray(fc_w, np.float32).reshape(4, 128)
    ys = []
    for c in range(N_CORES):
        p = res.results[c]["pout"]  # [4, 128, BC]
        ys.append(np.einsum('cp,cpb->b', fcw, p.astype(np.float32)))
    out = np.concatenate(ys).reshape(B, 1).astype(np.float32)
    out = out + np.asarray(fc_b, np.float32).reshape(1, 1)
    return out
